# revision 3
# baseline (speedup 1.0000x reference)
"""Bilateral-solver-3D loss kernel for 8 TRN2 NeuronCores.

Loss = n_pix*LAM*mean(w_ij * d^2) + mean((output-target)^2), where
d[k,t,h,w] = output[t,h,w] - xp[t+kt, h+i, w+j] over K=2204 offsets
(kt,i,j) of a 5x21x21 stencil (center removed), xp = edge-padded output.

Strategy (memory-bound: the 282MB w_ij stream dominates):
  - Shard spatially: core c owns h in [10c, 10c+10) for all t -> 50
    (t,h) pairs per core; every core sees all K offsets. SPMD-uniform
    program; only the data differs per core.
  - On-chip layout: partition axis = w (80 lanes), free axis = padded
    offset index kidx = i*110 + j*5 + kt (KPAD = 21*22*5 = 2310; the
    center and the j=21 columns carry w=0 so they contribute nothing).
  - Per (t,h) pair: ScalarE computes d2 = Square(-xs + x) in ONE op,
    reading xs directly as a strided window view of a per-partition
    sliding-window tensor xps[w, (j,tp,hp)] = xp[tp, hp, w+j] (host
    prepared, bf16) with per-partition bias x[t,h,w]. VectorE does one
    bf16 2x tensor_tensor multiply m2 = w * d2. TensorE contracts the
    partition axis with a ones column into PSUM, accumulating across
    all 50 pairs. Tiny final reduce + scale on device; the host adds
    the 8 per-core scalars.
  - w is quantized to bf16 on the host (sum rel-err ~1e-6, way inside
    tolerance) halving HBM traffic.
"""

import os
import sys

import numpy as np

_TRN_REPO = "/opt/trn_rl_repo"
if _TRN_REPO not in sys.path:
    sys.path.insert(0, _TRN_REPO)

# ---- problem geometry (hardcoded per contract) ----
T, H, W = 5, 80, 80
TK, SK = 5, 21
CT, CS = 2, 10
LAM = 128.0
KTRUE = 2204
NI, NJ, NKT = 21, 22, 5          # i window, j window (incl. dead j=21), kt
KPAD = NI * NJ * NKT             # 2310
NCORES = 8
HB = H // NCORES                 # 10 h-rows per core
PAIRS = T * HB                   # 50 (t, h_local) pairs per core
TP = T + 2 * CT                  # 9  padded T
HPW = HB + 2 * CS                # 30 padded-h window height per core
WP1 = W + 2 * CS + 1             # 101 padded W (+1 col for the j=21 reads)
XPS_FREE = NJ * TP * HPW         # 5940 elements per partition
GP = 10                          # pairs per w-DMA chunk
NB = 5                           # PSUM bank chunks of KPAD
KB = KPAD // NB                  # 462
N_PIX = T * H * W                # 32000
FID_P, FID_F = 128, N_PIX // 128  # fidelity tile (128, 250)

LAST_RESULTS = None  # BassKernelResults of the most recent run (for test.py)

_CACHE = {}


def _offsets():
    offs = [
        (k, i, j)
        for i in range(SK)
        for j in range(SK)
        for k in range(TK)
        if not (i == CS and j == CS and k == CT)
    ]
    assert len(offs) == KTRUE
    return offs


def _build_nc():
    import concourse.bass as bass
    import concourse.mybir as mybir
    import concourse.tile as tile

    # -- walrus workaround: this container's walrus rejects any instruction
    # carrying >1 sync-wait and any drain resetting a multi-sem range
    # ("Too many sync wait commands"). Chunk resets; split waits onto
    # single-wait NOPs inserted before the instruction.
    def _chunked_dma_reset(self, semaphore_range=None):
        if semaphore_range is None:
            semaphore_range = self.bass._kernel_sem_range
        out = None
        for s in list(semaphore_range):
            out = self.drain(semaphore_range=range(s, s + 1))
        return out

    bass.BassGpSimd.dma_reset = _chunked_dma_reset

    def _split_multi_waits(nc):
        n_split = 0
        for f in nc.m.functions:
            for bb in f.blocks:
                insts = list(bb.instructions)
                out = []
                changed = False
                for ins in insts:
                    si = ins.sync_info
                    if si is not None and len(si.on_wait) > 1:
                        waits = list(si.on_wait)
                        for wi, wct in enumerate(waits[:-1]):
                            nop = mybir.InstNoOp(
                                name=f"{ins.name}-w{wi}",
                                sync_info=mybir.SyncInfo(
                                    on_wait=[wct], on_update=[]
                                ),
                                bass_nofuse=True,
                                engine=ins.engine,
                            )
                            nc.register_instruction(nop, overwrite=True)
                            out.append(nop)
                        ins.sync_info = mybir.SyncInfo(
                            on_wait=[waits[-1]], on_update=list(si.on_update)
                        )
                        changed = True
                        n_split += 1
                    out.append(ins)
                if changed:
                    bb.instructions = out
        return n_split

    bf16 = mybir.dt.bfloat16
    f32 = mybir.dt.float32

    nc = bass.Bass()
    w_d = nc.dram_tensor("w", [W, PAIRS * KPAD], bf16, kind="ExternalInput")
    xps_d = nc.dram_tensor("xps", [W, XPS_FREE], bf16, kind="ExternalInput")
    xc_d = nc.dram_tensor("xc", [W, PAIRS], bf16, kind="ExternalInput")
    xf_d = nc.dram_tensor("xf", [FID_P, FID_F], f32, kind="ExternalInput")
    tf_d = nc.dram_tensor("tf", [FID_P, FID_F], f32, kind="ExternalInput")
    out_d = nc.dram_tensor("out", [1, 1], f32, kind="ExternalOutput")

    def win_view(ap, dims, extra_off):
        """Custom strided (overlapping) view of an SBUF tile AP."""
        v = ap.copy()
        p0 = v.ap[0]
        v.ap = mybir.VecI64Pair([list(p0)] + [list(d) for d in dims])
        v.offset = v.offset + extra_off
        return v

    with tile.TileContext(nc) as tc:
        with (
            tc.tile_pool(name="const", bufs=1) as cpool,
            tc.tile_pool(name="wbuf", bufs=2) as wpool,
            tc.tile_pool(name="d2buf", bufs=3) as d2pool,
            tc.tile_pool(name="m2buf", bufs=3) as m2pool,
            tc.tile_pool(name="psum", bufs=1, space="PSUM") as psum_pool,
        ):
            xps = cpool.tile([W, XPS_FREE], bf16)
            nc.sync.dma_start(xps[:], xps_d[:])
            xc = cpool.tile([W, PAIRS], bf16)
            nc.sync.dma_start(xc[:], xc_d[:])
            ones80 = cpool.tile([W, 1], bf16)
            nc.vector.memset(ones80[:], 1.0)

            ps = psum_pool.tile([1, NB, 512], f32)

            for g in range(PAIRS // GP):
                wt = wpool.tile([W, GP * KPAD], bf16)
                nc.sync.dma_start(
                    wt[:], w_d[:, g * GP * KPAD : (g + 1) * GP * KPAD]
                )
                for pl in range(GP):
                    p = g * GP + pl
                    t, hl = p // HB, p % HB
                    d2 = d2pool.tile([W, KPAD], bf16)
                    xs = win_view(
                        xps[:],
                        [[1, NI], [TP * HPW, NJ], [HPW, NKT]],
                        t * HPW + hl,
                    )
                    nc.scalar.activation(
                        d2[:],
                        xs,
                        mybir.ActivationFunctionType.Square,
                        bias=xc[:, p : p + 1],
                        scale=-1.0,
                    )
                    m2 = m2pool.tile([W, KPAD], bf16)
                    nc.vector.tensor_tensor(
                        m2[:],
                        wt[:, pl * KPAD : (pl + 1) * KPAD],
                        d2[:],
                        op=mybir.AluOpType.mult,
                    )
                    for b in range(NB):
                        nc.tensor.matmul(
                            ps[0:1, b, 0:KB],
                            ones80[:],
                            m2[:, b * KB : (b + 1) * KB],
                            start=(p == 0),
                            stop=(p == PAIRS - 1),
                        )

            # ---- final reduction of the smooth term ----
            s5 = cpool.tile([1, NB, KB], f32)
            nc.vector.tensor_copy(s5[:], ps[0:1, :, 0:KB])
            stot = cpool.tile([1, 1], f32)
            nc.vector.reduce_sum(stot[:], s5[:], axis=mybir.AxisListType.XY)

            # ---- fidelity term (identical on every core; host sums /8) ----
            fx = cpool.tile([FID_P, FID_F], f32)
            nc.sync.dma_start(fx[:], xf_d[:])
            ft = cpool.tile([FID_P, FID_F], f32)
            nc.sync.dma_start(ft[:], tf_d[:])
            fd = cpool.tile([FID_P, FID_F], f32)
            nc.vector.tensor_tensor(
                fd[:], fx[:], ft[:], op=mybir.AluOpType.subtract
            )
            fsq = cpool.tile([FID_P, FID_F], f32)
            nc.scalar.square(fsq[:], fd[:])
            frow = cpool.tile([FID_P, 1], f32)
            nc.vector.reduce_sum(frow[:], fsq[:], axis=mybir.AxisListType.X)
            ones128 = cpool.tile([FID_P, 1], f32)
            nc.vector.memset(ones128[:], 1.0)
            psf = psum_pool.tile([1, 1], f32)
            nc.tensor.matmul(psf[:], ones128[:], frow[:], start=True, stop=True)

            # ---- combine: out = stot*LAM/KTRUE + fid/(NCORES*n_pix) ----
            r1 = cpool.tile([1, 1], f32)
            nc.vector.tensor_scalar_mul(r1[:], stot[:], LAM / KTRUE)
            r2 = cpool.tile([1, 1], f32)
            nc.vector.tensor_scalar_mul(r2[:], psf[:], 1.0 / (NCORES * N_PIX))
            res = cpool.tile([1, 1], f32)
            nc.vector.tensor_tensor(
                res[:], r1[:], r2[:], op=mybir.AluOpType.add
            )
            nc.sync.dma_start(out_d[:], res[:])

    _split_multi_waits(nc)
    return nc


def _prep_inputs(w_ij, target, output):
    import ml_dtypes

    bf16 = ml_dtypes.bfloat16
    x = np.ascontiguousarray(output, dtype=np.float32)
    tgt = np.ascontiguousarray(target, dtype=np.float32)

    # padded volume with one extra w column for the dead j=21 reads
    xp = np.pad(x, ((CT, CT), (CS, CS), (CS, CS)), mode="edge")
    xp101 = np.concatenate([xp, xp[:, :, -1:]], axis=2)  # (9, 100, 101)
    xpb = xp101.astype(bf16)

    # sliding window over w+j: sw[tp, hp, w, j] = xpb[tp, hp, w+j]
    sw = np.lib.stride_tricks.sliding_window_view(xpb, NJ, axis=2)
    assert sw.shape == (TP, 2 * CS + H, W, NJ)

    xb3 = x.astype(bf16)  # (T, H, W)

    # w reorder: arr[w, t, h, n] then scatter n -> kidx
    offs = _offsets()
    kidx = np.array([i * (NJ * NKT) + j * NKT + k for (k, i, j) in offs])
    arr = np.ascontiguousarray(
        np.asarray(w_ij, dtype=np.float32).transpose(3, 1, 2, 0)
    ).astype(bf16)  # (W, T, H, KTRUE)

    xf = x.reshape(FID_P, FID_F)
    tf = tgt.reshape(FID_P, FID_F)

    in_maps = []
    for c in range(NCORES):
        h0 = HB * c
        w_re = np.zeros((W, T, HB, KPAD), dtype=bf16)
        w_re[:, :, :, kidx] = arr[:, :, h0 : h0 + HB, :]
        xps_c = np.ascontiguousarray(
            sw[:, h0 : h0 + HPW, :, :].transpose(2, 3, 0, 1)
        )  # (W, NJ, TP, HPW)
        xc_c = np.ascontiguousarray(
            xb3[:, h0 : h0 + HB, :].transpose(2, 0, 1)
        )  # (W, T, HB)
        in_maps.append(
            {
                "w": w_re.reshape(W, PAIRS * KPAD),
                "xps": xps_c.reshape(W, XPS_FREE),
                "xc": xc_c.reshape(W, PAIRS),
                "xf": xf,
                "tf": tf,
            }
        )
    return in_maps


def kernel(w_ij, target, output):
    global LAST_RESULTS
    from concourse.bass_utils import run_bass_kernel_spmd

    if "nc" not in _CACHE:
        _CACHE["nc"] = _build_nc()
    nc = _CACHE["nc"]

    in_maps = _prep_inputs(w_ij, target, output)
    r = run_bass_kernel_spmd(nc, in_maps, core_ids=list(range(NCORES)))
    LAST_RESULTS = r
    total = np.float32(0.0)
    for c in range(NCORES):
        total = total + np.float32(r.results[c]["out"][0, 0])
    return np.asarray(total, dtype=np.float32)
